# revision 7
# baseline (speedup 1.0000x reference)
"""CFSDP (density-peaks clustering) on 8 Trainium2 NeuronCores.

Pipeline (N=8192 points, D=64, row-sharded 1024 rows/core):
  d2(i,j) = ||xi-xj||^2 via one K=66 augmented matmul per tile:
      u_i = (-2*x_i, sq_i, 1),  v_j = (x_j, 1, sq_j),  d2 = u_i . v_j
  All O(N^2) math runs on squared distances (sqrt is monotone, so order
  stats / argmin / percentile commute with it):
    L1: count(d2 < t_b) for 16 thresholds around the predicted 2%-quantile
        (ACT sigmoid step fn + accumulate) -> host interpolates dc^2.
    L2: rho_i = sum_j exp(-d2_ij/dc^2) (ACT Exp + accumulate, scale from SBUF).
    host: stable-sort rows by rho desc; "higher density" mask becomes a
        per-row prefix of the sorted column order.
    L3: delta_i^2 = min over prefix window of d2 (vector.tensor_mask_reduce,
        per-partition index window, on negated-d2 PSUM tiles).
  Host finishes: delta fallback (row max) for top-density rows, nhd argmin
  (lazy, only for non-center points), center ranks, label propagation scan.
"""

import os
import numpy as np

N = 8192
D = 64
NCORES = 8
ROWS = N // NCORES          # 1024 rows per core
P = 128                     # partitions
RB = ROWS // P              # 8 row-blocks per core
FD = 2048                   # free-dim group (4 PSUM banks)
G = N // FD                 # 4 col-groups per row
K = D + 4                   # 68 (augmented contraction dim, sq split hi+lo)
MM_N = 512                  # cols per matmul (one PSUM bank output)
MM_PER_G = FD // MM_N       # 4

NT = 4                      # percentile-count thresholds
L1_W = 1024                 # cols counted per threshold
DC2_CENTER = 86.2           # chi^2_64-predicted 2%-quantile of d2 (randn data)
DC2_GRID = (DC2_CENTER * (1.0 + (np.arange(NT) - (NT - 1) / 2) * 0.023)).astype(
    np.float64
)                           # +-3.5% bracket, 2.3% spacing
SIG_ALPHA = 2.0e4           # sigmoid step sharpness (soft window ~1e-3 in d2)
PCT = 2.0
FLT_MAX = float(np.finfo(np.float32).max)
PEN_BIG = 1e38              # penalty added beyond the prefix cutoff
PEN_ALPHA = 1e31            # relu penalty slope (ACT-built mask, L3)
WW = 1024                   # L3 boundary mask window width
NCOL = G + 1                # L3 output cols per block (G group slots + window)
EMPTY_SENTINEL = 1e37       # accum >= this => empty prefix window

# threshold b is counted on group (m, g) of every core (1/16 of the matrix
# per threshold => ~4.2M samples each; different rows+cols per threshold)
L1_GROUPS = [(b % RB, 1 + b % (G - 1)) for b in range(NT)]  # g>0: diag-free
DC2_STEP = float(DC2_CENTER * 0.023)
M_TOT = float(N) * float(N)
K_POS = PCT / 100.0 * (M_TOT - 1.0)
P_OFF = (K_POS - N) / (M_TOT - N)      # diag-free target CDF
CSTAR = float(P_OFF * P * L1_W)        # target count over the device sample

_programs: dict = {}


def _f32r(ap):
    import concourse.mybir as mybir

    return ap.bitcast(mybir.dt.float32r)



def _pe_warmup(nc, tc, inp, psum_p, mybir, n_mm=8):
    """Dense garbage-matmul burst at launch start: runs while the input DMA
    streams, trips the PE HAM un-throttle (~3.4us sustained busy) so real
    matmuls run at 2.4 GHz instead of 1.2 GHz."""
    f32 = mybir.dt.float32
    warm = inp.tile([K, P + MM_N], mybir.dt.bfloat16)
    nc.gpsimd.memset(warm[:], 1.0)
    wps = psum_p.tile([P, FD], f32, tag="psum")
    for j in range(n_mm):
        nc.tensor.matmul(
            wps[:, (j % MM_PER_G) * MM_N:((j % MM_PER_G) + 1) * MM_N],
            warm[:, :P],
            warm[:, P:P + MM_N],
            start=True,
            stop=True,
        )


def _build_l12():
    """Merged count + rho launch: dc^2 is computed ON DEVICE.

    Every core counts the SAME sample (rows 0..1023 via the shared `uvc`
    lhsT, diag-free col groups), so each core independently derives an
    identical dc^2 - no collectives. The CDF interpolation runs as tiny
    [1,8] vector ops; a PE ones-matmul does the partition reduction and a
    K=1 fp32 matmul broadcasts -1/dc^2 to all partitions for the rho phase.
    `dvec` carries host-computed control-variate corrections (in counts)
    that cancel the row/col sampling bias of the fixed sample.
    """
    import concourse.mybir as mybir
    import concourse.tile as tile
    from concourse import bacc

    f32 = mybir.dt.float32
    nc = bacc.Bacc("TRN2", debug=False, enable_asserts=False)
    bf16 = mybir.dt.bfloat16
    uv_d = nc.dram_tensor("uv", [K, ROWS + N], bf16, kind="ExternalInput")
    uvc_d = nc.dram_tensor("uvc", [K, ROWS], bf16, kind="ExternalInput")
    thr_d = nc.dram_tensor("thr", [P, NT], f32, kind="ExternalInput")
    tvec_d = nc.dram_tensor("tvec", [1, NT], f32, kind="ExternalInput")
    dvec_d = nc.dram_tensor("dvec", [1, NT], f32, kind="ExternalInput")
    cnt_d = nc.dram_tensor("counts", [P, NT], f32, kind="ExternalOutput")
    rho_d = nc.dram_tensor("rho", [P, RB], f32, kind="ExternalOutput")

    with tile.TileContext(nc) as tc:
        with (
            tc.tile_pool(name="inp", bufs=1) as inp,
            tc.tile_pool(name="stat", bufs=1) as stat,
            tc.tile_pool(name="trash", bufs=2) as trash_p,
            tc.tile_pool(name="psum", bufs=2, space="PSUM") as psum_p,
        ):
            uv_sb = inp.tile([K, ROWS + N], bf16)
            uvc_sb = inp.tile([K, ROWS], bf16)
            nc.sync.dma_start(out=uvc_sb[:], in_=uvc_d[:])
            for _g in (1, 2, 3):  # count-phase cols first; group 0 only for rho
                _a = ROWS + _g * FD
                nc.sync.dma_start(
                    out=uv_sb[:, _a:_a + FD], in_=uv_d[:, _a:_a + FD]
                )
            nc.sync.dma_start(out=uv_sb[:, 0:ROWS], in_=uv_d[:, 0:ROWS])
            nc.sync.dma_start(
                out=uv_sb[:, ROWS:ROWS + FD], in_=uv_d[:, ROWS:ROWS + FD]
            )
            thr_sb = inp.tile([P, NT], f32)
            nc.gpsimd.dma_start(out=thr_sb[:], in_=thr_d[:])
            tdv_sb = inp.tile([1, 2 * NT], f32)
            nc.gpsimd.dma_start(out=tdv_sb[:, 0:NT], in_=tvec_d[:])
            nc.gpsimd.dma_start(out=tdv_sb[:, NT:2 * NT], in_=dvec_d[:])
            cnts = stat.tile([P, NT], f32)
            warmact = stat.tile([P, 1], f32)
            nc.scalar.activation(
                warmact[:], thr_sb[:, 0:1],
                mybir.ActivationFunctionType.Sigmoid, bias=0.0, scale=1.0,
            )

            # ---- phase 1: counts over the shared sample -----------------
            for b, (m, g) in enumerate(L1_GROUPS):
                psum = psum_p.tile([P, FD], f32, tag="psum")
                for j in range(L1_W // MM_N):
                    nc.tensor.matmul(
                        psum[:, j * MM_N:(j + 1) * MM_N],
                        uvc_sb[:, m * P:(m + 1) * P],
                        uv_sb[:, ROWS + g * FD + j * MM_N: ROWS + g * FD + (j + 1) * MM_N],
                        start=True,
                        stop=True,
                    )
                t = trash_p.tile([P, L1_W], f32, tag="cntrash")
                nc.scalar.activation(
                    t[:],
                    psum[:, 0:L1_W],
                    mybir.ActivationFunctionType.Sigmoid,
                    bias=thr_sb[:, b:b + 1],
                    scale=float(-SIG_ALPHA),
                    accum_out=cnts[:, b:b + 1],
                )
            nc.gpsimd.dma_start(out=cnt_d[:], in_=cnts[:])

            # ---- phase 2: dc^2 from counts (identical on every core) ----
            ones_col = stat.tile([P, 1], f32)
            nc.vector.memset(ones_col[:], 1.0)
            ps_tot = psum_p.tile([1, NT], f32, tag="psum")
            nc.tensor.matmul(ps_tot[:], ones_col[:], cnts[:], start=True, stop=True)
            w = stat.tile([1, 8 * NT], f32)  # scratch lanes along free dim
            q = w[:, 0:NT]
            nc.vector.tensor_tensor(
                out=q, in0=ps_tot[:], in1=tdv_sb[:, NT:2 * NT],
                op=mybir.AluOpType.subtract,
            )
            NB_ = NT - 1
            a_ = w[:, NT:NT + NB_]
            nc.vector.tensor_scalar(
                out=a_, in0=q[:, 0:NB_], scalar1=CSTAR, scalar2=None,
                op0=mybir.AluOpType.is_le,
            )
            b_ = w[:, 2 * NT:2 * NT + NB_]
            nc.vector.tensor_scalar(
                out=b_, in0=q[:, 1:NT], scalar1=CSTAR, scalar2=None,
                op0=mybir.AluOpType.is_gt,
            )
            sel = w[:, 3 * NT:3 * NT + NB_]
            nc.vector.tensor_tensor(out=sel, in0=a_, in1=b_, op=mybir.AluOpType.mult)
            den = w[:, 4 * NT:4 * NT + NB_]
            nc.vector.tensor_tensor(
                out=den, in0=q[:, 1:NT], in1=q[:, 0:NB_],
                op=mybir.AluOpType.subtract,
            )
            rec = w[:, 5 * NT:5 * NT + NB_]
            nc.vector.reciprocal(rec, den)
            num = w[:, 6 * NT:6 * NT + NB_]
            nc.vector.tensor_scalar(
                out=num, in0=q[:, 0:NB_], scalar1=-1.0, scalar2=CSTAR,
                op0=mybir.AluOpType.mult, op1=mybir.AluOpType.add,
            )
            fr = w[:, 7 * NT:7 * NT + NB_]
            nc.vector.tensor_tensor(out=fr, in0=num, in1=rec, op=mybir.AluOpType.mult)
            nc.vector.tensor_scalar(
                out=fr, in0=fr, scalar1=float(DC2_STEP), scalar2=None,
                op0=mybir.AluOpType.mult,
            )
            nc.vector.tensor_tensor(
                out=fr, in0=fr, in1=tdv_sb[:, 0:NB_], op=mybir.AluOpType.add
            )
            nc.vector.tensor_tensor(out=fr, in0=fr, in1=sel, op=mybir.AluOpType.mult)
            sc = stat.tile([1, 4], f32)
            nc.vector.tensor_reduce(
                sc[:, 0:1], fr[:], axis=mybir.AxisListType.X, op=mybir.AluOpType.add
            )
            nc.vector.tensor_reduce(
                sc[:, 1:2], sel[:], axis=mybir.AxisListType.X, op=mybir.AluOpType.add
            )
            # guard: if no bracket, fall back to the grid center
            nc.vector.tensor_scalar(
                out=sc[:, 2:3], in0=sc[:, 1:2], scalar1=float(-DC2_CENTER),
                scalar2=float(DC2_CENTER), op0=mybir.AluOpType.mult,
                op1=mybir.AluOpType.add,
            )
            nc.vector.tensor_tensor(
                out=sc[:, 0:1], in0=sc[:, 0:1], in1=sc[:, 2:3],
                op=mybir.AluOpType.add,
            )
            nc.vector.reciprocal(sc[:, 3:4], sc[:, 0:1])
            nc.vector.tensor_scalar(
                out=sc[:, 3:4], in0=sc[:, 3:4], scalar1=-1.0, scalar2=None,
                op0=mybir.AluOpType.mult,
            )
            ones_row = stat.tile([1, P], f32)
            nc.vector.memset(ones_row[:], 1.0)
            ps_b = psum_p.tile([P, 1], f32, tag="psum")
            nc.tensor.matmul(ps_b[:], ones_row[:], sc[:, 3:4], start=True, stop=True)
            scl_sb = stat.tile([P, 1], f32)
            nc.vector.tensor_copy(scl_sb[:], ps_b[:])

            # ---- phase 3: rho ------------------------------------------
            parts = stat.tile([P, RB * G], f32)
            rho_sb = stat.tile([P, RB], f32)
            for m in range(RB):
                for g in range(G):
                    psum = psum_p.tile([P, FD], f32, tag="psum")
                    for j in range(MM_PER_G):
                        nc.tensor.matmul(
                            psum[:, j * MM_N:(j + 1) * MM_N],
                            uv_sb[:, m * P:(m + 1) * P],
                            uv_sb[:, ROWS + g * FD + j * MM_N: ROWS + g * FD + (j + 1) * MM_N],
                            start=True,
                            stop=True,
                        )
                    t = trash_p.tile([P, FD], f32, tag="trash")
                    q2 = m * G + g
                    nc.scalar.activation(
                        t[:],
                        psum[:],
                        mybir.ActivationFunctionType.Exp,
                        bias=0.0,
                        scale=scl_sb[:, 0:1],
                        accum_out=parts[:, q2:q2 + 1],
                    )
                nc.vector.tensor_reduce(
                    rho_sb[:, m:m + 1],
                    parts[:, m * G:(m + 1) * G],
                    axis=mybir.AxisListType.X,
                    op=mybir.AluOpType.add,
                )
            nc.sync.dma_start(out=rho_d[:], in_=rho_sb[:])
    nc.compile()
    return nc


def _build_l3():
    """Delta pass on rho-sorted data (round-robin block interleaving).

    Core c holds sorted row-blocks b = 8m + c (m = 0..7). For local block m:
      boundary col-group g_b = m//2, window base w_lo = 1024*(m%2)
      (cutoffs of every core's block-m rows lie in [w_lo, w_lo+1024) of
      group g_b, ties aside - those are patched on host).
    Structure per block:
      groups g < g_b:                plain min-reduce of the whole group
      boundary prefix [0, w_lo):     plain min-reduce (odd m only)
      boundary window [w_lo,+1024):  penalty mask (iota >= cutrel)*BIG, add,
                                     min-reduce
      columns beyond w_lo+1024 and groups g > g_b: skipped entirely.
    """
    import concourse.mybir as mybir
    import concourse.tile as tile
    from concourse import bacc

    f32 = mybir.dt.float32
    nc = bacc.Bacc("TRN2", debug=False, enable_asserts=False)
    bf16 = mybir.dt.bfloat16
    uv_d = nc.dram_tensor("uv", [K, ROWS + N], bf16, kind="ExternalInput")
    cut_d = nc.dram_tensor("cut", [P, RB], f32, kind="ExternalInput")
    iota_d = nc.dram_tensor("iota", [P, WW], f32, kind="ExternalInput")
    dmin_d = nc.dram_tensor("dmin", [P, RB * NCOL], f32, kind="ExternalOutput")

    with tile.TileContext(nc) as tc:
        with (
            tc.tile_pool(name="inp", bufs=1) as inp,
            tc.tile_pool(name="stat", bufs=1) as stat,
            tc.tile_pool(name="trash", bufs=3) as trash_p,
            tc.tile_pool(name="pen", bufs=3) as pen_p,
            tc.tile_pool(name="psum", bufs=2, space="PSUM") as psum_p,
        ):
            uv_sb = inp.tile([K, ROWS + N], bf16)
            nc.sync.dma_start(out=uv_sb[:, 0:ROWS], in_=uv_d[:, 0:ROWS])
            for _g in range(G):
                _a = ROWS + _g * FD
                nc.sync.dma_start(
                    out=uv_sb[:, _a:_a + FD], in_=uv_d[:, _a:_a + FD]
                )
            cut_sb = inp.tile([P, RB], f32)
            nc.gpsimd.dma_start(out=cut_sb[:], in_=cut_d[:])
            iota_sb = inp.tile([P, WW], f32)
            nc.gpsimd.dma_start(out=iota_sb[:], in_=iota_d[:])
            dmin_sb = stat.tile([P, RB * NCOL], f32)

            for m in range(RB):
                gb = m // 2
                w_lo = WW * (m % 2)
                pen = pen_p.tile([P, WW], f32, tag="pen")
                # cutrel (host-clamped to [0, WW]) is relative to w_lo
                nc.vector.tensor_scalar(
                    out=pen[:],
                    in0=iota_sb[:],
                    scalar1=cut_sb[:, m:m + 1],
                    scalar2=PEN_BIG,
                    op0=mybir.AluOpType.is_ge,
                    op1=mybir.AluOpType.mult,
                )
                for g in range(gb + 1):
                    ncols = FD if g < gb else w_lo + WW
                    psum = psum_p.tile([P, FD], f32, tag="psum")
                    for j in range(ncols // MM_N):
                        nc.tensor.matmul(
                            psum[:, j * MM_N:(j + 1) * MM_N],
                            uv_sb[:, m * P:(m + 1) * P],
                            uv_sb[:, ROWS + g * FD + j * MM_N: ROWS + g * FD + (j + 1) * MM_N],
                            start=True,
                            stop=True,
                        )
                    q = m * NCOL + g
                    if g < gb:
                        nc.vector.tensor_reduce(
                            dmin_sb[:, q:q + 1],
                            psum[:],
                            axis=mybir.AxisListType.X,
                            op=mybir.AluOpType.min,
                        )
                    else:
                        if w_lo > 0:
                            nc.vector.tensor_reduce(
                                dmin_sb[:, q:q + 1],
                                psum[:, 0:w_lo],
                                axis=mybir.AxisListType.X,
                                op=mybir.AluOpType.min,
                            )
                        t = trash_p.tile([P, WW], f32, tag="trash")
                        nc.vector.tensor_tensor(
                            out=t[:],
                            in0=psum[:, w_lo:w_lo + WW],
                            in1=pen[:],
                            op=mybir.AluOpType.add,
                        )
                        nc.vector.tensor_reduce(
                            dmin_sb[:, m * NCOL + G:m * NCOL + G + 1],
                            t[:],
                            axis=mybir.AxisListType.X,
                            op=mybir.AluOpType.min,
                        )
            nc.gpsimd.dma_start(out=dmin_d[:], in_=dmin_sb[:])
    nc.compile()
    return nc


CK = 66                     # cert GEMM contraction: 64 x-dims + sq hi/lo
C_NACT = 10                 # ACT (exp-detector) tiles per core
C_NDVE = 10                 # DVE (min-reduce) tiles per core
CERT_EPS = 0.30             # bf16 augmented-GEMM d2 error bound (measured .28)
CERT_SMAX = 0.35            # exp-detector flag threshold (signal >= .76)
C_FILL2048 = 2              # PE filler matmuls per 2048 tile (p-state hold)
C_FILL1024 = 1              # PE filler matmuls per 1024 tile
C_NWARM = 6                 # PE warmup matmuls bridging the input DMA

# (m, g0, w, eng): local block m (global rows 128*(8m+c)) covers global cols
# [1024m, 8192); coverage is cut into 2048-col tiles (plus 1024 tails), each
# consumed WHOLE by one engine: 'A' = ACT exp detector, 'V' = DVE min-reduce.
# Every block's lowest-col tile holds its diagonal run ([1024m+128c, +128)
# for every core) and must be 'A'. Order alternates engines and descends in
# column start so tiles become ready as the descending-chunk DMA of V lands.
C_TILES = [
    (7, 7168, 1024, "A"),
    (5, 7168, 1024, "V"),
    (6, 6144, 2048, "A"),
    (3, 7168, 1024, "V"),
    (5, 5120, 2048, "A"),
    (4, 6144, 2048, "V"),
    (1, 7168, 1024, "A"),
    (3, 5120, 2048, "V"),
    (4, 4096, 2048, "A"),
    (2, 6144, 2048, "V"),
    (3, 3072, 2048, "A"),
    (1, 5120, 2048, "V"),
    (2, 2048, 2048, "A"),
    (2, 4096, 2048, "V"),
    (0, 4096, 2048, "A"),
    (1, 3072, 2048, "V"),
    (1, 1024, 2048, "A"),
    (0, 6144, 2048, "V"),
    (0, 0, 2048, "A"),
    (0, 2048, 2048, "V"),
]


def _plan_cert():
    """Tile schedule with output-column bookkeeping (shared builder/host)."""
    tiles = []
    s_i = 0
    v_i = 0
    for m, g0, w, eng in C_TILES:
        if eng == "A":
            tiles.append(dict(m=m, g0=g0, w=w, eng=eng, col=s_i))
            s_i += 1
        else:
            tiles.append(dict(m=m, g0=g0, w=w, eng=eng, col=v_i))
            v_i += 1
    assert s_i == C_NACT and v_i == C_NDVE
    return tiles


def _build_cert():
    """Single-launch close-pair certificate: per-row min d2 (DVE segments)
    plus a sum-of-exp(-d2/2) detector (ACT segments) over the block-upper
    triangle. PSUM holds sq_j - 2 xi.xj; sq_i enters via the ACT bias or on
    host after the min reduce."""
    import concourse.mybir as mybir
    import concourse.tile as tile
    from concourse import bacc

    f32 = mybir.dt.float32
    bf16 = mybir.dt.bfloat16
    nc = bacc.Bacc("TRN2", debug=False, enable_asserts=False)
    u_d = nc.dram_tensor("u", [CK, ROWS], bf16, kind="ExternalInput")
    v_d = nc.dram_tensor("v", [CK, N], bf16, kind="ExternalInput")
    bias_d = nc.dram_tensor("bias", [P, RB], f32, kind="ExternalInput")
    s_d = nc.dram_tensor("s", [P, C_NACT], f32, kind="ExternalOutput")
    m_d = nc.dram_tensor("m", [P, C_NDVE], f32, kind="ExternalOutput")

    with tile.TileContext(nc) as tc:
        with (
            tc.tile_pool(name="inp", bufs=1) as inp,
            tc.tile_pool(name="stat", bufs=1) as stat,
            tc.tile_pool(name="trash", bufs=2) as trash_p,
            tc.tile_pool(name="psum", bufs=2, space="PSUM") as psum_p,
        ):
            u_sb = inp.tile([CK, ROWS], bf16)
            v_sb = inp.tile([CK, N], bf16)
            bias_sb = inp.tile([P, RB], f32)
            # warm operands via Vector (gpsimd stalls on its preamble drain)
            warm = inp.tile([CK, P + MM_N], bf16)
            nc.vector.memset(warm[:], 1.0)
            wact = stat.tile([P, 2], f32)
            nc.vector.memset(wact[:, 0:1], 0.0)
            nc.scalar.activation(
                wact[:, 1:2], wact[:, 0:1],
                mybir.ActivationFunctionType.Exp, bias=0.0, scale=1.0,
            )
            nc.gpsimd.dma_start(out=bias_sb[:], in_=bias_d[:])
            nc.sync.dma_start(out=u_sb[:], in_=u_d[:])
            for ch in range(7, -1, -1):
                nc.sync.dma_start(
                    out=v_sb[:, ch * 1024:(ch + 1) * 1024],
                    in_=v_d[:, ch * 1024:(ch + 1) * 1024],
                )
            s_sb = stat.tile([P, C_NACT], f32)
            m_sb = stat.tile([P, C_NDVE], f32)

            def fill_mm(psum):
                # garbage matmul into the tile's first bank; discarded by the
                # real start=True write. Keeps the PE stream gapless so the
                # p-state ramp to 2.4 GHz survives consumer-bound stretches.
                nc.tensor.matmul(
                    psum[:, 0:MM_N],
                    warm[:, :P],
                    warm[:, P:P + MM_N],
                    start=True,
                    stop=True,
                )

            wps = psum_p.tile([P, FD], f32, tag="psum")
            for j in range(C_NWARM):
                nc.tensor.matmul(
                    wps[:, (j % MM_PER_G) * MM_N:((j % MM_PER_G) + 1) * MM_N],
                    warm[:, :P],
                    warm[:, P:P + MM_N],
                    start=True,
                    stop=True,
                )

            for t in _plan_cert():
                m, g0, w, col = t["m"], t["g0"], t["w"], t["col"]
                psum = psum_p.tile([P, FD], f32, tag="psum")
                for _ in range(C_FILL2048 if w == 2048 else C_FILL1024):
                    fill_mm(psum)
                for j in range(w // MM_N):
                    nc.tensor.matmul(
                        psum[:, j * MM_N:(j + 1) * MM_N],
                        u_sb[:, m * P:(m + 1) * P],
                        v_sb[:, g0 + j * MM_N:g0 + (j + 1) * MM_N],
                        start=True,
                        stop=True,
                    )
                if t["eng"] == "A":
                    tr = trash_p.tile([P, FD], bf16, tag="trash")
                    nc.scalar.activation(
                        tr[:, 0:w],
                        psum[:, 0:w],
                        mybir.ActivationFunctionType.Exp,
                        bias=bias_sb[:, m:m + 1],
                        scale=-0.5,
                        accum_out=s_sb[:, col:col + 1],
                    )
                else:
                    nc.vector.tensor_reduce(
                        m_sb[:, col:col + 1],
                        psum[:, 0:w],
                        axis=mybir.AxisListType.X,
                        op=mybir.AluOpType.min,
                    )
            nc.sync.dma_start(out=s_d[:], in_=s_sb[:])
            nc.sync.dma_start(out=m_d[:], in_=m_sb[:])
    nc.compile()
    return nc


_BUILDERS = {"l12": _build_l12, "l3": _build_l3, "cert": _build_cert}


def _get_program(name):
    if name not in _programs:
        _programs[name] = _BUILDERS[name]()
    return _programs[name]


TIMINGS = []  # (name, exec_time_ns) per launch, appended by _run


def _run(name, in_maps, trace=None):
    from concourse.bass_utils import run_bass_kernel_spmd

    if trace is None:
        trace = bool(int(os.environ.get("KERNEL_TRACE", "0")))
    nc = _get_program(name)
    res = run_bass_kernel_spmd(
        nc, in_maps, core_ids=list(range(NCORES)), trace=trace
    )
    TIMINGS.append((name, res.exec_time_ns))
    return res


def _augmented(data):
    """U (lhs rows) and V (rhs cols) of the K=68 augmented distance GEMM.

    bf16 operands with sq split into a bf16 hi+lo pair: d2 error ~0.04 abs
    (~5e-4 relative at the dc^2 scale), far inside every decision margin.
    """
    import ml_dtypes

    bf = ml_dtypes.bfloat16
    sq = np.einsum("ij,ij->i", data, data, dtype=np.float32).astype(np.float32)
    sqh = sq.astype(bf)
    sql = (sq - sqh.astype(np.float32)).astype(bf)
    ones = np.ones((N, 1), bf)
    zcol = lambda a: a[:, None]
    U = np.concatenate(
        [(-2.0 * data).astype(bf), zcol(sqh), zcol(sql), ones, ones], axis=1
    )
    V = np.concatenate(
        [data.astype(bf), ones, ones, zcol(sqh), zcol(sql)], axis=1
    )
    return U, V, sq


def _erf(x):
    """Abramowitz-Stegun 7.1.26 vectorized erf (|err| < 1.5e-7)."""
    s = np.sign(x)
    x = np.abs(x)
    t = 1.0 / (1.0 + 0.3275911 * x)
    y = 1.0 - (
        ((((1.061405429 * t - 1.453152027) * t) + 1.421413741) * t - 0.284496736)
        * t
        + 0.254829592
    ) * t * np.exp(-x * x)
    return s * y


def _phi(z):
    return 0.5 * (1.0 + _erf(z / np.sqrt(2.0)))


NGRID = 256


def _cv_corrections(sq):
    """Control-variate count corrections for the fixed count sample.

    Model P(d2 < t | sq_i, sq_j) ~ Phi((t - sq_i - sq_j)/(2 sqrt(sq_i sq_j/D)))
    and subtract the predicted row/col selection bias of the sampled
    rows/cols relative to the full point set.
    """
    sq64 = sq.astype(np.float64)
    step = N // NGRID
    grid = np.sort(sq64)[step // 2::step][:NGRID]

    def h(t, svals):
        s = svals[:, None]
        sp = grid[None, :]
        z = (t - s - sp) / (2.0 * np.sqrt(np.maximum(s * sp, 1e-9) / D))
        return _phi(z).mean(axis=1)

    dvec = np.zeros(NT)
    for b, (m, g) in enumerate(L1_GROUPS):
        t = float(DC2_GRID[b])
        h_all = h(t, grid).mean()
        d_row = h(t, sq64[m * P:(m + 1) * P]).mean() - h_all
        d_col = h(t, sq64[g * FD:g * FD + L1_W]).mean() - h_all
        dvec[b] = (d_row + d_col) * (P * L1_W)
    return dvec.astype(np.float32).reshape(1, NT)


def _interp_dc2(counts_by_core):
    """counts_by_core: list of [P, NT] arrays -> dc^2 via CDF interpolation."""
    M = float(N) * float(N)
    k_pos = PCT / 100.0 * (M - 1.0)
    p_off = (k_pos - N) / (M - N)  # diag cells (d2=0) all fall below any t_b

    tot = np.zeros(NT, np.float64)
    denom = np.zeros(NT, np.float64)
    for c in range(NCORES):
        cc = counts_by_core[c].astype(np.float64).sum(axis=0)  # [NT]
        for b, (m, g) in enumerate(L1_GROUPS):
            row0 = c * ROWS + m * P
            off = row0 - g * FD
            has_diag = 0 <= off <= L1_W - P
            tot[b] += cc[b] - (P if has_diag else 0)
            denom[b] += P * L1_W - (P if has_diag else 0)
    p_hat = tot / denom
    # p_hat should be increasing in b; enforce monotonicity for safety
    p_mono = np.maximum.accumulate(p_hat)
    if not (p_mono[0] <= p_off <= p_mono[-1]):
        return None  # bracket miss -> caller falls back to exact host path
    b_hi = int(np.searchsorted(p_mono, p_off, side="left"))
    if b_hi == 0:
        return float(DC2_GRID[0])
    b_lo = b_hi - 1
    p_lo, p_hi_v = p_mono[b_lo], p_mono[b_hi]
    frac = 0.0 if p_hi_v <= p_lo else (p_off - p_lo) / (p_hi_v - p_lo)
    return float(DC2_GRID[b_lo] + frac * (DC2_GRID[b_hi] - DC2_GRID[b_lo]))


def _host_fallback(data, rho_t, delta_t):
    """Pure-numpy reference path (only used if device assumptions break)."""
    data = np.asarray(data, np.float32)
    sq = np.sum(data * data, axis=1)
    d2 = sq[:, None] + sq[None, :] - 2.0 * (data @ data.T)
    dist = np.sqrt(np.maximum(d2, 0.0), dtype=np.float32)
    dc = np.percentile(dist, PCT)
    rho = np.exp(-((dist / dc) ** 2)).sum(axis=1).astype(np.float32)
    higher = rho[None, :] > rho[:, None]
    masked = np.where(higher, dist, np.inf)
    delta_m = masked.min(axis=1)
    nhd_m = masked.argmin(axis=1)
    has = higher.any(axis=1)
    delta = np.where(has, delta_m, dist.max(axis=1))
    nhd = np.where(has, nhd_m, np.arange(N))
    return _finish_labels(rho, delta, nhd, rho_t, delta_t)


def _finish_labels(rho, delta, nhd, rho_t, delta_t):
    is_center = (rho > rho_t) & (delta > delta_t)
    center_rank = np.cumsum(is_center.astype(np.int32)) - 1
    labels = np.where(is_center, center_rank, -1).astype(np.int32)
    order = np.argsort(-rho, kind="stable")
    for i in order:
        if labels[i] < 0:
            labels[i] = labels[nhd[i]]
    return labels


def kernel(data, rho_threshold, delta_threshold):
    data = np.ascontiguousarray(np.asarray(data, dtype=np.float32))
    assert data.shape == (N, D)
    rho_t = float(np.asarray(rho_threshold))
    delta_t = float(np.asarray(delta_threshold))

    lab = _kernel_cert(data, rho_t, delta_t)
    if lab is not None:
        return lab
    return _kernel_full(data, rho_t, delta_t)


def _kernel_cert(data, rho_t, delta_t):
    """All-centers fast path.

    Device computes, over every unordered point pair, either an exact
    (bf16-accurate) min of d2 or a sum-of-exp(-d2/2) close-pair detector.
    If no pair is closer than delta_threshold (plus error margin) and the
    rho lower bound 1 + (N-1)exp(-d2max/dc^2) clears rho_threshold, then
    every point satisfies rho > rho_t and delta > delta_t, making every
    point a cluster center: labels == arange exactly. Returns None when
    the certificate does not hold (caller falls back to the full path).
    """
    import ml_dtypes

    bf = ml_dtypes.bfloat16
    sq = np.einsum("ij,ij->i", data, data, dtype=np.float32).astype(np.float32)
    sqh = sq.astype(bf)
    sql = (sq - sqh.astype(np.float32)).astype(bf)
    v_in = np.ascontiguousarray(
        np.concatenate(
            [data.astype(bf).T, sqh[None, :], sql[None, :]], axis=0
        )
    )  # [CK, N]
    in_maps = []
    for c in range(NCORES):
        ucols = []
        bias = np.empty((P, RB), np.float32)
        for m in range(RB):
            b = 8 * m + c
            rows = slice(b * P, (b + 1) * P)
            blk = np.concatenate(
                [
                    (-2.0 * data[rows]).astype(bf),
                    np.ones((P, 2), bf),
                ],
                axis=1,
            )  # [P, CK]
            ucols.append(blk.T)
            bias[:, m] = -0.5 * sq[rows]
        in_maps.append(
            {
                "u": np.ascontiguousarray(np.concatenate(ucols, axis=1)),
                "v": v_in,
                "bias": bias,
            }
        )
    r = _run("cert", in_maps)

    plan = _plan_cert()
    m_glob = np.inf
    s_res_max = -np.inf
    for c in range(NCORES):
        S = r.results[c]["s"].astype(np.float64)  # [P, C_NACT]
        M = r.results[c]["m"]  # [P, C_NDVE]
        for m in range(RB):
            rows = slice((8 * m + c) * P, (8 * m + c + 1) * P)
            scols = [t["col"] for t in plan if t["m"] == m and t["eng"] == "A"]
            vcols = [t["col"] for t in plan if t["m"] == m and t["eng"] == "V"]
            s_res = S[:, scols].sum(axis=1) - 1.0  # diag term removed
            s_res_max = max(s_res_max, float(s_res.max()))
            if vcols:
                mv = M[:, vcols].min(axis=1) + sq[rows]
                m_glob = min(m_glob, float(mv.min()))

    if not np.isfinite(m_glob) or not np.isfinite(s_res_max):
        return None
    if s_res_max >= CERT_SMAX:
        return None
    # DVE-covered pairs have true d2 > m_glob - eps; ACT-covered pairs
    # (un-flagged detector) have exp(-d2_meas/2) < SMAX + diag-resid,
    # i.e. true d2 > -2 ln(SMAX + .131) - eps > 1.16. Global bound:
    d2_lo = min(m_glob - CERT_EPS, 1.1)
    if d2_lo <= 0.0:
        return None
    # delta_i >= min_j dist > delta_t for every i
    if delta_t >= 0.0 and d2_lo <= delta_t * delta_t:
        return None
    # rho_i >= 1 + (N-1) exp(-d2max / dc^2), dc^2 >= d2_lo,
    # d2max <= (2 max|x|)^2 exactly on host
    d2max = float(4.0 * sq.astype(np.float64).max())
    rho_lb = 1.0 + 0.9 * (N - 1) * float(np.exp(-d2max / d2_lo))
    if rho_t >= rho_lb:
        return None
    return np.arange(N, dtype=np.int32)


def _kernel_full(data, rho_t, delta_t):
    U, V, sq = _augmented(data)
    VT = V.T  # [K, N]

    # ---- L12: counts -> on-device dc^2 -> rho (single launch) ----------
    thr = np.broadcast_to(
        (SIG_ALPHA * DC2_GRID).astype(np.float32)[None, :], (P, NT)
    ).copy()
    tvec = DC2_GRID.astype(np.float32).reshape(1, NT)
    dvec = _cv_corrections(sq)
    uvc = np.ascontiguousarray(np.concatenate([U[0:ROWS].T, VT], axis=1)[:, 0:ROWS])
    in_maps = [
        {
            "uv": np.ascontiguousarray(
                np.concatenate([U[c * ROWS:(c + 1) * ROWS].T, VT], axis=1)
            ),
            "uvc": uvc,
            "thr": thr,
            "tvec": tvec,
            "dvec": dvec,
        }
        for c in range(NCORES)
    ]
    r12 = _run("l12", in_maps)

    # validate the on-device dc interpolation from the counts output
    q = r12.results[0]["counts"].astype(np.float64).sum(axis=0) - dvec[0].astype(
        np.float64
    )
    brackets = [
        b for b in range(NT - 1) if q[b] <= CSTAR < q[b + 1]
    ]
    if len(brackets) != 1 or not np.all(np.diff(q) > 0):
        return _host_fallback(data, rho_t, delta_t)

    rho = np.empty(N, np.float32)
    for c in range(NCORES):
        out = r12.results[c]["rho"]  # [P, RB]
        rho[c * ROWS:(c + 1) * ROWS] = out.T.reshape(-1)
    if not np.all(np.isfinite(rho)) or rho.min() < 0.5 or rho.max() > N + 1:
        return _host_fallback(data, rho_t, delta_t)

    # ---- host: sort by rho desc; prefix cutoffs ------------------------
    order = np.argsort(-rho, kind="stable")
    rho_sorted = rho[order]
    # c_i = #points with rho strictly greater (ties excluded)
    cuts = np.searchsorted(-rho_sorted, -rho_sorted, side="left").astype(np.int64)

    data_p = data[order]
    sq_p = sq[order]
    Up = U[order]
    Vp = V[order]
    rhs_p = np.ascontiguousarray(Vp.T)

    # round-robin block interleave: core c <- sorted blocks 8m + c
    NB = N // P  # 64 sorted row-blocks
    blk_rows = np.arange(N).reshape(NB, P)
    core_rows = [blk_rows[np.arange(RB) * NCORES + c].reshape(-1) for c in range(NCORES)]

    iota_in = np.broadcast_to(
        np.arange(WW, dtype=np.float32)[None, :], (P, WW)
    ).copy()
    in_maps = []
    for c in range(NCORES):
        rows = core_rows[c]
        cutrel = np.empty((P, RB), np.float32)
        for m in range(RB):
            base = (m // 2) * FD + WW * (m % 2)
            cutrel[:, m] = np.clip(cuts[rows[m * P:(m + 1) * P]] - base, 0, WW)
        in_maps.append(
            {
                "uv": np.ascontiguousarray(
                    np.concatenate([Up[rows].T, rhs_p], axis=1)
                ),
                "cut": cutrel,
                "iota": iota_in,
            }
        )
    r3 = _run("l3", in_maps)
    # dmin[i] holds per-source minima; dcol[k] = (col_base, col_len) of source k
    dmin = np.full((N, NCOL), np.inf, np.float32)
    for c in range(NCORES):
        out = r3.results[c]["dmin"]  # [P, RB*NCOL]
        rows = core_rows[c]
        for m in range(RB):
            gb = m // 2
            w_lo = WW * (m % 2)
            blk = rows[m * P:(m + 1) * P]
            for g in range(gb):
                dmin[blk, g] = out[:, m * NCOL + g]
            if w_lo > 0:
                dmin[blk, gb] = out[:, m * NCOL + gb]
            dmin[blk, G] = out[:, m * NCOL + G]

    # ---- host: delta, fallback rows, centers, nhd (lazy), labels -------
    delta2_sorted = dmin.min(axis=1)

    # rho-tie rows whose cutoff dips below their block's boundary group: the
    # device's full-group reduce included a few extra columns; fix exactly.
    win_base = ((np.arange(N) // P) // NCORES) * WW  # 1024*m per sorted row
    straddle_fix = {}
    for i in np.nonzero(cuts < win_base)[0]:
        cut = int(cuts[i])
        if cut == 0:
            delta2_sorted[i] = np.inf
            continue
        d2row = sq_p[i] + sq_p[:cut] - 2.0 * (data_p[:cut] @ data_p[i])
        j = int(np.argmin(d2row))
        delta2_sorted[i] = d2row[j]
        straddle_fix[i] = j

    empty = delta2_sorted >= EMPTY_SENTINEL  # no higher-density point
    delta_sorted = np.sqrt(np.maximum(delta2_sorted, 0.0), dtype=np.float32)
    for i in np.nonzero(empty)[0]:
        d2row = sq_p[i] + sq_p - 2.0 * (data_p @ data_p[i])
        delta_sorted[i] = np.sqrt(max(float(np.max(np.maximum(d2row, 0.0))), 0.0))

    delta = np.empty(N, np.float32)
    delta[order] = delta_sorted

    is_center = (rho > rho_t) & (delta > delta_t)
    center_rank = np.cumsum(is_center.astype(np.int32)) - 1
    labels = np.where(is_center, center_rank, -1).astype(np.int32)

    need_nhd = ~is_center[order]  # sorted positions whose label must propagate
    nhd = np.arange(N, dtype=np.int64)  # default: self (matches reference)
    for i in np.nonzero(need_nhd)[0]:
        if empty[i]:
            continue  # nhd stays self, as in reference
        if i in straddle_fix:
            nhd[order[i]] = order[straddle_fix[i]]
            continue
        k = int(np.argmin(dmin[i]))
        m = (i // P) // NCORES
        gb = m // 2
        w_lo = WW * (m % 2)
        if k == G:
            c0, clen = gb * FD + w_lo, WW
        elif k == gb:
            c0, clen = gb * FD, w_lo
        else:
            c0, clen = k * FD, FD
        end_local = int(np.clip(cuts[i] - c0, 0, clen))
        cols = slice(c0, c0 + end_local)
        d2part = sq_p[i] + sq_p[cols] - 2.0 * (data_p[cols] @ data_p[i])
        j_local = int(np.argmin(d2part))
        nhd[order[i]] = order[c0 + j_local]

    for i in order:
        if labels[i] < 0:
            labels[i] = labels[nhd[i]]
    return labels.astype(np.int32)

